# revision 11
# baseline (speedup 1.0000x reference)
"""Trainium2 Bass kernel for nn_Net_73710228734901.

The network's post-gather graph (concat -> Conv3d -> spatial mean -> Linear)
is entirely linear in the gathered pixels, and the gathers / avg-pool /
1x1-conv are linear in the inputs.  Since the output is only [B, 1], the
whole model collapses to

    out[b] = lin_b + <W1, x1[b]> + <W2, x2[b]> + <W4, share[b]> + <W3, x3[b]>

with fixed per-element weight tensors W* computed (cheaply, on host) from
c_w / conv3d_w / lin_w / idx_h / idx_w.  The device kernel is then a pure
memory-bound weighted reduction over the big activations.

Key structure (vs the first working version):
  1. The x1/x2/share weights are nonzero only on each channel's 7x7 crop
     window (the gather), so the host ships just the 49 cropped pixels per
     channel instead of all 196 -- a 4x traffic cut on those tensors.
  2. The reduction is split across THREE engines so none is the
     bottleneck and the kernel runs at the DMA roofline:
       - DVE + ScalarE, batch-major stream xa: 4 batches are packed per
         instruction (each batch's 128 channels folded onto 32
         partitions) so the per-instruction overheads are paid 16x not
         64x.  Some groups run as one fused STT (DVE only, 1x mode);
         the rest as TT (DVE 2x mode) + activation-accumulate (ScalarE).
       - PE, batch-minor stream xb: the first NKPE of x3's 980
         per-partition columns as a PSUM-accumulated chunked matvec:
         psum[1, 64] += w3[:, c].T @ x3[:, c, :].
  3. Per-batch sums of the packed groups are recovered with a tiny
     selector matmul; the host does the final unscale + bias + cross-core
     sum (it already has to sum the 8 cores' partials).

Sharding: channels are sharded 8 ways (x1/x2/share: 128 ch/core, x3:
160 ch/core); every core holds all 64 batches and produces partial sums.
Per-core HBM traffic ~18.8 MB fp16.
"""

import numpy as np

import concourse.bacc as bacc
import concourse.mybir as mybir
from concourse.bass_utils import run_bass_kernel_spmd
from concourse.tile import TileContext

NCORES = 8
NB = 64           # full batch, all on every core (channel sharding)
FCROP = 147       # 3 * 49 cropped pixels (x1/x2/share) per channel
F3 = 980          # x3 shard: 160 ch * 784 pos / 128 partitions
NKPE = 448        # x3 per-partition columns handled by the tensor engine
NSTRIP = 4        # PE column-strips used round-robin (concurrent col tiles)
FD = 680          # per-batch free dim: 147 + (980-NKPE) padded
NPACK = 8         # batches packed per DVE/ACT instruction
NG = NB // NPACK  # 8 packed groups
NPART = 128 // NPACK  # partitions per packed batch
ACT_GROUPS = 5    # of the 8 groups, this many take the TT+ACT path
GBLK = 1          # groups per xa DMA block (8 batches -> 8 rounds)
BC = 64           # PE columns per xb DMA block (NKPE/BC = 7 rounds)
ABUFS = 4         # xa double-buffer depth
BBUFS = 5         # xb double-buffer depth
W_SCALE = 1024.0  # weights pre-scaled by 2^10 so fp16 products avoid
                  # subnormals; undone exactly in the host combine

_F32 = mybir.dt.float32
_F16 = mybir.dt.float16


def _build_fold(c_w, conv3d_w, lin_w, lin_b, idx_h, idx_w):
    """Collapse conv3d+mean+linear into per-element weights (float64 host math).

    Returns Wc1, Wc2, Wc4: [1024, 49] crop-window weights and
    Ws3: [1280, 784] float32 (x3 pulled back through 1x1 conv + avg-pool).
    """
    c_w = c_w.astype(np.float64)
    conv3d_w = conv3d_w.astype(np.float64)
    lin_w = lin_w.astype(np.float64)

    # W2[c = i*64+dd, kh, kw] = sum_{o,d,kd: 3d-4+kd=dd} lin_w[o*24+d] * conv3d_w[o,i,kd,kh,kw]
    W2 = np.zeros((1024, 3, 3), np.float64)
    o_idx = np.arange(32) * 24
    i_idx = np.arange(16) * 64
    for d in range(24):
        for kd in range(3):
            dd = 3 * d - 4 + kd
            if 0 <= dd < 64:
                W2[i_idx + dd] += np.einsum(
                    'o,oikl->ikl', lin_w[o_idx + d, 0], conv3d_w[:, :, kd])

    # Mean over the 14x14 conv output folds each (kh,kw) tap into a border mask.
    M = np.zeros((3, 3, 14, 14), np.float64)
    rng = {0: (0, 13), 1: (0, 14), 2: (1, 14)}
    for kh in range(3):
        for kw in range(3):
            r0, r1 = rng[kh]
            c0, c1 = rng[kw]
            M[kh, kw, r0:r1, c0:c1] = 1.0
    A = np.einsum('ckl,klrs->crs', W2, M) / 196.0   # [1024, 14, 14]

    # Quadrant weights apply directly to the cropped 7x7 windows.
    Wc1 = A[:, 0:7, 0:7].reshape(1024, 49)
    Wc2 = A[:, 7:14, 0:7].reshape(1024, 49)
    Wc4 = A[:, 7:14, 7:14].reshape(1024, 49)

    # x3 path: scatter quadrant 3 to the 14x14 grid (inverse of the gather),
    # pull back through the 1x1 conv, then through avg_pool2d(5, 2, 2).
    Ws3c = np.zeros((1024, 14, 14), np.float64)
    ci = np.arange(1024)[:, None, None]
    ri = (idx_h[2][:, None] + np.arange(7))[:, :, None]
    wi = (idx_w[2][:, None] + np.arange(7))[:, None, :]
    Ws3c[ci, ri, wi] = A[:, 0:7, 7:14]
    Wpool = np.einsum('oc,ohw->chw', c_w, Ws3c)     # [1280, 14, 14]
    Ws3 = np.zeros((1280, 28, 28), np.float64)
    for dh in range(-2, 3):
        for dw in range(-2, 3):
            hs = [h for h in range(14) if 0 <= 2 * h + dh < 28]
            ws = [w for w in range(14) if 0 <= 2 * w + dw < 28]
            H = [2 * h + dh for h in hs]
            W_ = [2 * w + dw for w in ws]
            Ws3[:, np.ix_(H, W_)[0], np.ix_(H, W_)[1]] += \
                Wpool[:, np.ix_(hs, ws)[0], np.ix_(hs, ws)[1]] / 25.0

    return (Wc1.astype(np.float32), Wc2.astype(np.float32),
            Wc4.astype(np.float32), Ws3.reshape(1280, 784).astype(np.float32))


def _crop_gather(x, ih, iw):
    """x: [B, 1024, 14, 14]; per-channel 7x7 crops -> [B, 1024, 49]."""
    B = x.shape[0]
    bi = np.arange(B)[:, None, None, None]
    ci = np.arange(1024)[None, :, None, None]
    ri = (ih[:, None] + np.arange(7))[None, :, :, None]
    wi = (iw[:, None] + np.arange(7))[None, :, None, :]
    return x[bi, ci, ri, wi].reshape(B, 1024, 49)


def _on_act(g):
    return (g * ACT_GROUPS) % NG < ACT_GROUPS


def _build_bass(nkpe=NKPE, fd=FD, gblk=GBLK, bc=BC, abufs=ABUFS, bbufs=BBUFS):
    """DMA-roofline weighted reduction split across DVE, ScalarE and PE.

    Streams two fp16 tensors per core:
      xa [128, NG, 4*FD]  4-batch-packed batch-major (batch 4g+q's channels
                          live on partitions [32q, 32q+32)); one STT or
                          TT+ACT per group accumulates into acc[:, g].
      xb [128, NKPE, NB]  batch-minor; one tiny matmul per column
                          accumulates w3[:, c].T @ xb[:, c, :] into
                          pe_ps[1, NB] across all NKPE columns.
    Outputs two raw partial-sum tensors (selector-matmul of acc, and the
    PE psum); the host finishes unscale + bias + cross-core reduction.
    """
    nc = bacc.Bacc("TRN2")
    fp = NPACK * fd
    xa = nc.dram_tensor("xa", [128, NG, fp], _F16, kind="ExternalInput")
    xb = nc.dram_tensor("xb", [128, nkpe, NB], _F16, kind="ExternalInput")
    wa = nc.dram_tensor("wa", [128, fp], _F16, kind="ExternalInput")
    wb = nc.dram_tensor("wb", [128, nkpe], _F16, kind="ExternalInput")
    seli = nc.dram_tensor("seli", [128, NPACK], _F32, kind="ExternalInput")
    outa = nc.dram_tensor("outa", [NPACK, NG], _F32, kind="ExternalOutput")
    outb = nc.dram_tensor("outb", [NSTRIP, NB], _F32, kind="ExternalOutput")

    ra = NG // gblk      # xa rounds
    rb = nkpe // bc      # xb rounds
    rounds = max(ra, rb)

    with TileContext(nc) as tc:
        with (
            tc.tile_pool(name="cpool", bufs=1) as cpool,
            tc.tile_pool(name="xapool", bufs=abufs) as xapool,
            tc.tile_pool(name="xbpool", bufs=bbufs) as xbpool,
            tc.tile_pool(name="spool", bufs=2) as spool,
            tc.tile_pool(name="gpool", bufs=2) as gpool,
            tc.tile_pool(name="apool", bufs=1) as apool,
            tc.tile_pool(name="ppool", bufs=1, space="PSUM") as ppool,
        ):
            # First data blocks go out before the (small) weight loads so
            # the DMA queues are saturated from t=0.
            xa_t0 = xapool.tile([128, gblk, fp], _F16, tag="xa")
            nc.sync.dma_start(out=xa_t0[:], in_=xa[:, 0:gblk, :])
            xb_t0 = xbpool.tile([128, bc, NB], _F16, tag="xb")
            nc.sync.dma_start(out=xb_t0[:], in_=xb[:, 0:bc, :])

            wa_t = cpool.tile([128, fp], _F16)
            nc.sync.dma_start(out=wa_t[:], in_=wa[:, :])
            wb_t = cpool.tile([128, nkpe], _F16)
            nc.sync.dma_start(out=wb_t[:], in_=wb[:, :])
            # Selector for recovering per-batch sums from the packed acc:
            # sel[p, q] = 1 iff p // NPART == q (host-built constant).
            sel = cpool.tile([128, NPACK], _F32)
            nc.sync.dma_start(out=sel[:], in_=seli[:, :])

            acc = apool.tile([128, NG], _F32)
            # PE partials: strip j accumulates on psum partition 32*j so the
            # four column-strips of the array run concurrently.
            pe_ps = ppool.tile([128, NB], _F32)

            for r in range(rounds):
                if r < ra:
                    if r == 0:
                        xa_t = xa_t0
                    else:
                        xa_t = xapool.tile([128, gblk, fp], _F16, tag="xa")
                        nc.sync.dma_start(
                            out=xa_t[:], in_=xa[:, r * gblk:(r + 1) * gblk, :])
                if r < rb:
                    if r == 0:
                        xb_t = xb_t0
                    else:
                        xb_t = xbpool.tile([128, bc, NB], _F16, tag="xb")
                        nc.sync.dma_start(
                            out=xb_t[:], in_=xb[:, r * bc:(r + 1) * bc, :])
                if r < ra:
                    for j in range(gblk):
                        g = r * gblk + j
                        if _on_act(g):
                            prod = gpool.tile([128, fp], _F16, tag="prod")
                            nc.vector.tensor_tensor(
                                prod[:], xa_t[:, j, :], wa_t[:],
                                mybir.AluOpType.mult)
                            sink = gpool.tile([128, fp], _F16, tag="sink")
                            nc.scalar.activation(
                                sink[:], prod[:],
                                mybir.ActivationFunctionType.Copy,
                                accum_out=acc[:, g:g + 1])
                        else:
                            scr = spool.tile([128, fp], _F16, tag="scr")
                            nc.vector.scalar_tensor_tensor(
                                out=scr[:],
                                in0=xa_t[:, j, :],
                                scalar=1.0,
                                in1=wa_t[:],
                                op0=mybir.AluOpType.mult,
                                op1=mybir.AluOpType.mult,
                                accum_out=acc[:, g:g + 1],
                            )
                if r < rb:
                    for k in range(bc):
                        c = r * bc + k
                        j = c % NSTRIP
                        nc.tensor.matmul(
                            pe_ps[32 * j:32 * j + 1, :],
                            lhsT=wb_t[:, c:c + 1],
                            rhs=xb_t[:, k, :],
                            start=(c < NSTRIP),
                            stop=(c >= nkpe - NSTRIP),
                            tile_position=(0, 32 * j),
                        )

            # Per-batch sums of the packed groups: sel.T @ acc -> [8, 8].
            sel_ps = ppool.tile([NPACK, NG], _F32)
            nc.tensor.matmul(sel_ps[:, :], lhsT=sel[:], rhs=acc[:],
                             start=True, stop=True)
            res_a = apool.tile([NPACK, NG], _F32)
            nc.vector.tensor_copy(res_a[:], sel_ps[:, :])
            # The 4 strip rows live on partitions {0, 32, 64, 96}; copy each
            # to SBUF in place and ship them with one partition-strided DMA.
            res_b = apool.tile([128, NB], _F32)
            for j in range(NSTRIP):
                nc.vector.tensor_copy(
                    res_b[32 * j:32 * j + 1, :], pe_ps[32 * j:32 * j + 1, :])
            nc.sync.dma_start(out=outa[:, :], in_=res_a[:])
            nc.sync.dma_start(out=outb[:, :], in_=res_b[0:97:32, :])
    nc.finalize()
    return nc


def _shard_inputs(x1, x2, x3, share_feature, Wc1, Wc2, Wc4, Ws3, lin_b,
                  idx_h, idx_w, nkpe=NKPE, fd=FD):
    xc1 = _crop_gather(x1, idx_h[0], idx_w[0])            # [64, 1024, 49]
    xc2 = _crop_gather(x2, idx_h[1], idx_w[1])
    xc4 = _crop_gather(share_feature, idx_h[3], idx_w[3])
    x3f = x3.reshape(NB, 1280, 784)

    npad = fd - (FCROP + (F3 - nkpe))
    in_maps = []
    for m in range(NCORES):
        cs = slice(m * 128, (m + 1) * 128)
        cs3 = slice(m * 160, (m + 1) * 160)
        x3s = x3f[:, cs3].reshape(NB, 128, F3)            # [64, 128, 980]
        w3s = Ws3[cs3].reshape(128, F3)                   # [128, 980]

        xaf = np.concatenate([
            xc1[:, cs], xc2[:, cs], xc4[:, cs],
            x3s[:, :, nkpe:],
            np.zeros((NB, 128, npad), np.float32),
        ], axis=2)                                        # [64, 128, FD]
        # pack: batch b = NPACK*g+q, channel c = NPART*k+r ->
        # xa[NPART*q+r, g, k*FD+f] = xaf[NPACK*g+q, NPART*k+r, f]
        xa = xaf.reshape(NG, NPACK, NPACK, NPART, fd).transpose(1, 3, 0, 2, 4)
        xa = np.ascontiguousarray(
            xa.reshape(128, NG, NPACK * fd), dtype=np.float16)
        xb = np.ascontiguousarray(
            x3s[:, :, :nkpe].transpose(1, 2, 0), dtype=np.float16)

        waf = np.concatenate([
            Wc1[cs], Wc2[cs], Wc4[cs],
            w3s[:, nkpe:],
            np.zeros((128, npad), np.float32),
        ], axis=1) * W_SCALE                              # [128, FD]
        # wa[NPART*q+r, k*FD+f] = waf[NPART*k+r, f]  (independent of q)
        wav = np.tile(
            waf.reshape(NPACK, NPART, fd).transpose(1, 0, 2).reshape(NPART, -1),
            (NPACK, 1))
        wbv = w3s[:, :nkpe] * W_SCALE                     # [128, NKPE]

        selv = np.zeros((128, NPACK), np.float32)
        for q in range(NPACK):
            selv[NPART * q:NPART * (q + 1), q] = 1.0
        in_maps.append({
            'xa': xa,
            'xb': xb,
            'wa': np.ascontiguousarray(wav, dtype=np.float16),
            'wb': np.ascontiguousarray(wbv, dtype=np.float16),
            'seli': selv,
        })
    return in_maps


def _combine(results, lin_b):
    """Host-side finish: per-core partials -> [64, 1] fp32 output."""
    tot = np.zeros(NB, np.float64)
    for r in results:
        a = np.asarray(r['outa'], np.float64)             # [NPACK, NG] (q, g)
        b = np.asarray(r['outb'], np.float64).sum(axis=0)  # strips -> [64]
        tot += a.T.ravel() + b                            # b = NPACK*g+q order
    tot = tot / W_SCALE + float(lin_b[0])
    return tot.astype(np.float32).reshape(NB, 1)


def _ensure_ntff_hook():
    """Make `trace=True` (e.g. BASS_TRACE=1) work under axon even when the
    image's antenv package lacks axon_hooks: register an equivalent module
    backed by the ctypes NTFF hook from trn_agent_boot."""
    import sys
    import types
    try:
        import antenv.axon_hooks  # noqa: F401
        return
    except Exception:
        pass
    try:
        from trn_agent_boot import trn_boot
        hook = trn_boot._ntff_profile_via_ctypes('/opt/axon/libaxon_pjrt.so')
        mod = types.ModuleType('antenv.axon_hooks')
        mod.get_axon_ntff_profile_hook = lambda: hook
        mod.set_axon_ntff_profile_hook = lambda h: None
        sys.modules['antenv.axon_hooks'] = mod
    except Exception:
        pass


def kernel(x1, x2, x3, share_feature, c_w, conv3d_w, lin_w, lin_b,
           idx_h, idx_w):
    x1, x2, x3 = np.asarray(x1), np.asarray(x2), np.asarray(x3)
    share_feature = np.asarray(share_feature)
    c_w, conv3d_w = np.asarray(c_w), np.asarray(conv3d_w)
    lin_w, lin_b = np.asarray(lin_w), np.asarray(lin_b)
    idx_h, idx_w = np.asarray(idx_h), np.asarray(idx_w)
    _ensure_ntff_hook()
    Wc1, Wc2, Wc4, Ws3 = _build_fold(c_w, conv3d_w, lin_w, lin_b,
                                     idx_h, idx_w)
    in_maps = _shard_inputs(x1, x2, x3, share_feature,
                            Wc1, Wc2, Wc4, Ws3, lin_b, idx_h, idx_w)
    nc = _build_bass()
    res = run_bass_kernel_spmd(nc, in_maps, core_ids=list(range(NCORES)))
    return _combine(res.results, lin_b)


# revision 12
# speedup vs baseline: 1.0522x; 1.0522x over previous
"""Trainium2 Bass kernel for nn_Net_73710228734901.

The network's post-gather graph (concat -> Conv3d -> spatial mean -> Linear)
is entirely linear in the gathered pixels, and the gathers / avg-pool /
1x1-conv are linear in the inputs.  Since the output is only [B, 1], the
whole model collapses to

    out[b] = lin_b + <W1, x1[b]> + <W2, x2[b]> + <W4, share[b]> + <W3, x3[b]>

with fixed per-element weight tensors W* computed (cheaply, on host) from
c_w / conv3d_w / lin_w / idx_h / idx_w.  The device kernel is then a pure
memory-bound weighted reduction over the big activations.

Key structure (vs the first working version):
  1. The x1/x2/share weights are nonzero only on each channel's 7x7 crop
     window (the gather), so the host ships just the 49 cropped pixels per
     channel instead of all 196 -- a 4x traffic cut on those tensors.
  2. The reduction is split across THREE engines so none is the
     bottleneck and the kernel runs at the DMA roofline:
       - DVE + ScalarE, batch-major stream xa: 4 batches are packed per
         instruction (each batch's 128 channels folded onto 32
         partitions) so the per-instruction overheads are paid 16x not
         64x.  Some groups run as one fused STT (DVE only, 1x mode);
         the rest as TT (DVE 2x mode) + activation-accumulate (ScalarE).
       - PE, batch-minor stream xb: the first NKPE of x3's 980
         per-partition columns as a PSUM-accumulated chunked matvec:
         psum[1, 64] += w3[:, c].T @ x3[:, c, :].
  3. Per-batch sums of the packed groups are recovered with a tiny
     selector matmul; the host does the final unscale + bias + cross-core
     sum (it already has to sum the 8 cores' partials).

Sharding: channels are sharded 8 ways (x1/x2/share: 128 ch/core, x3:
160 ch/core); every core holds all 64 batches and produces partial sums.
Per-core HBM traffic ~18.8 MB fp16.
"""

import numpy as np

import concourse.bacc as bacc
import concourse.mybir as mybir
from concourse.bass_utils import run_bass_kernel_spmd
from concourse.tile import TileContext

NCORES = 8
NB = 64           # full batch, all on every core (channel sharding)
FCROP = 147       # 3 * 49 cropped pixels (x1/x2/share) per channel
F3 = 980          # x3 shard: 160 ch * 784 pos / 128 partitions
NKPE = 980        # x3 per-partition columns handled by the tensor engine
NSTRIP = 4        # PE column-strips used round-robin (concurrent col tiles)
FD = 148          # per-batch free dim: 147 crops padded to even
NPACK = 8         # batches packed per DVE instruction
NG = NB // NPACK  # 8 packed groups
NPART = 128 // NPACK  # partitions per packed batch
ACT_GROUPS = 0    # all groups on the fused-STT path (ScalarE left idle)
BC = 70           # PE columns per xb DMA block (NKPE/BC = 14 rounds)
BBUFS = 5         # xb double-buffer depth
W_SCALE = 1024.0  # weights pre-scaled by 2^10 so fp16 products avoid
                  # subnormals; undone exactly in the host combine

_F32 = mybir.dt.float32
_F16 = mybir.dt.float16


def _build_fold(c_w, conv3d_w, lin_w, lin_b, idx_h, idx_w):
    """Collapse conv3d+mean+linear into per-element weights (float64 host math).

    Returns Wc1, Wc2, Wc4: [1024, 49] crop-window weights and
    Ws3: [1280, 784] float32 (x3 pulled back through 1x1 conv + avg-pool).
    """
    c_w = c_w.astype(np.float64)
    conv3d_w = conv3d_w.astype(np.float64)
    lin_w = lin_w.astype(np.float64)

    # W2[c = i*64+dd, kh, kw] = sum_{o,d,kd: 3d-4+kd=dd} lin_w[o*24+d] * conv3d_w[o,i,kd,kh,kw]
    W2 = np.zeros((1024, 3, 3), np.float64)
    o_idx = np.arange(32) * 24
    i_idx = np.arange(16) * 64
    for d in range(24):
        for kd in range(3):
            dd = 3 * d - 4 + kd
            if 0 <= dd < 64:
                W2[i_idx + dd] += np.einsum(
                    'o,oikl->ikl', lin_w[o_idx + d, 0], conv3d_w[:, :, kd])

    # Mean over the 14x14 conv output folds each (kh,kw) tap into a border mask.
    M = np.zeros((3, 3, 14, 14), np.float64)
    rng = {0: (0, 13), 1: (0, 14), 2: (1, 14)}
    for kh in range(3):
        for kw in range(3):
            r0, r1 = rng[kh]
            c0, c1 = rng[kw]
            M[kh, kw, r0:r1, c0:c1] = 1.0
    A = np.einsum('ckl,klrs->crs', W2, M) / 196.0   # [1024, 14, 14]

    # Quadrant weights apply directly to the cropped 7x7 windows.
    Wc1 = A[:, 0:7, 0:7].reshape(1024, 49)
    Wc2 = A[:, 7:14, 0:7].reshape(1024, 49)
    Wc4 = A[:, 7:14, 7:14].reshape(1024, 49)

    # x3 path: scatter quadrant 3 to the 14x14 grid (inverse of the gather),
    # pull back through the 1x1 conv, then through avg_pool2d(5, 2, 2).
    Ws3c = np.zeros((1024, 14, 14), np.float64)
    ci = np.arange(1024)[:, None, None]
    ri = (idx_h[2][:, None] + np.arange(7))[:, :, None]
    wi = (idx_w[2][:, None] + np.arange(7))[:, None, :]
    Ws3c[ci, ri, wi] = A[:, 0:7, 7:14]
    Wpool = np.einsum('oc,ohw->chw', c_w, Ws3c)     # [1280, 14, 14]
    Ws3 = np.zeros((1280, 28, 28), np.float64)
    for dh in range(-2, 3):
        for dw in range(-2, 3):
            hs = [h for h in range(14) if 0 <= 2 * h + dh < 28]
            ws = [w for w in range(14) if 0 <= 2 * w + dw < 28]
            H = [2 * h + dh for h in hs]
            W_ = [2 * w + dw for w in ws]
            Ws3[:, np.ix_(H, W_)[0], np.ix_(H, W_)[1]] += \
                Wpool[:, np.ix_(hs, ws)[0], np.ix_(hs, ws)[1]] / 25.0

    return (Wc1.astype(np.float32), Wc2.astype(np.float32),
            Wc4.astype(np.float32), Ws3.reshape(1280, 784).astype(np.float32))


def _crop_gather(x, ih, iw):
    """x: [B, 1024, 14, 14]; per-channel 7x7 crops -> [B, 1024, 49]."""
    B = x.shape[0]
    bi = np.arange(B)[:, None, None, None]
    ci = np.arange(1024)[None, :, None, None]
    ri = (ih[:, None] + np.arange(7))[None, :, :, None]
    wi = (iw[:, None] + np.arange(7))[None, :, None, :]
    return x[bi, ci, ri, wi].reshape(B, 1024, 49)


def _on_act(g):
    return (g * ACT_GROUPS) % NG < ACT_GROUPS


def _build_bass(nkpe=NKPE, fd=FD, bc=BC, bbufs=BBUFS):
    """DMA-roofline weighted reduction: PE does x3, DVE does the crops.

    Streams two fp16 tensors per core:
      xa [128, NG, 8*FD]  8-batch-packed crops (batch 8g+q's channels live
                          on partitions [16q, 16q+16)); one fused STT per
                          group (mult + free-dim accum) into acc[:, g].
      xb [128, NKPE, NB]  x3 batch-minor; one tiny matmul per column:
                          strip j = c%4 accumulates w3[:, c].T @ x3[:, c, :]
                          on psum partition 32j, so the four column-strips
                          of the PE array run concurrently.
    Outputs raw partials (selector-matmul of acc -> [8, 8]; the 4 strip
    rows -> [4, 64]); the host finishes unscale + bias + reduction.
    """
    nc = bacc.Bacc("TRN2")
    fp = NPACK * fd
    xa = nc.dram_tensor("xa", [128, NG, fp], _F16, kind="ExternalInput")
    xb = nc.dram_tensor("xb", [128, nkpe, NB], _F16, kind="ExternalInput")
    wa = nc.dram_tensor("wa", [128, fp], _F16, kind="ExternalInput")
    wb = nc.dram_tensor("wb", [128, nkpe], _F16, kind="ExternalInput")
    seli = nc.dram_tensor("seli", [128, NPACK], _F32, kind="ExternalInput")
    outa = nc.dram_tensor("outa", [NPACK, NG], _F32, kind="ExternalOutput")
    outb = nc.dram_tensor("outb", [NSTRIP, NB], _F32, kind="ExternalOutput")

    rb = nkpe // bc      # xb rounds

    with TileContext(nc) as tc:
        with (
            tc.tile_pool(name="cpool", bufs=1) as cpool,
            tc.tile_pool(name="xbpool", bufs=bbufs) as xbpool,
            tc.tile_pool(name="spool", bufs=2) as spool,
            tc.tile_pool(name="apool", bufs=1) as apool,
            tc.tile_pool(name="ppool", bufs=1, space="PSUM") as ppool,
        ):
            # First xb block goes out first so the PE stream starts at t=0.
            xb_t0 = xbpool.tile([128, bc, NB], _F16, tag="xb")
            nc.sync.dma_start(out=xb_t0[:], in_=xb[:, 0:bc, :])
            # The whole (small) crops tensor in one DMA.
            xa_t = cpool.tile([128, NG, fp], _F16)
            nc.sync.dma_start(out=xa_t[:], in_=xa[:, :, :])
            wa_t = cpool.tile([128, fp], _F16)
            nc.sync.dma_start(out=wa_t[:], in_=wa[:, :])
            wb_t = cpool.tile([128, nkpe], _F16)
            nc.sync.dma_start(out=wb_t[:], in_=wb[:, :])
            sel = cpool.tile([128, NPACK], _F32)
            nc.sync.dma_start(out=sel[:], in_=seli[:, :])

            acc = apool.tile([128, NG], _F32)
            # PE partials: strip j accumulates on psum partition 32*j so the
            # four column-strips of the array run concurrently.
            pe_ps = ppool.tile([128, NB], _F32)

            for g in range(NG):
                scr = spool.tile([128, fp], _F16, tag="scr")
                # Fused multiply + free-dim sum in one DVE pass.
                nc.vector.scalar_tensor_tensor(
                    out=scr[:],
                    in0=xa_t[:, g, :],
                    scalar=1.0,
                    in1=wa_t[:],
                    op0=mybir.AluOpType.mult,
                    op1=mybir.AluOpType.mult,
                    accum_out=acc[:, g:g + 1],
                )

            for r in range(rb):
                if r == 0:
                    xb_t = xb_t0
                else:
                    xb_t = xbpool.tile([128, bc, NB], _F16, tag="xb")
                    nc.sync.dma_start(
                        out=xb_t[:], in_=xb[:, r * bc:(r + 1) * bc, :])
                for k in range(bc):
                    c = r * bc + k
                    j = c % NSTRIP
                    nc.tensor.matmul(
                        pe_ps[32 * j:32 * j + 1, :],
                        lhsT=wb_t[:, c:c + 1],
                        rhs=xb_t[:, k, :],
                        start=(c < NSTRIP),
                        stop=(c >= nkpe - NSTRIP),
                        tile_position=(0, 32 * j),
                    )

            # Per-batch sums of the packed groups: sel.T @ acc -> [8, 8].
            sel_ps = ppool.tile([NPACK, NG], _F32)
            nc.tensor.matmul(sel_ps[:, :], lhsT=sel[:], rhs=acc[:],
                             start=True, stop=True)
            res_a = apool.tile([NPACK, NG], _F32)
            nc.vector.tensor_copy(res_a[:], sel_ps[:, :])
            # The 4 strip rows live on partitions {0, 32, 64, 96}; copy each
            # to SBUF in place and ship them with one partition-strided DMA.
            res_b = apool.tile([128, NB], _F32)
            for j in range(NSTRIP):
                nc.vector.tensor_copy(
                    res_b[32 * j:32 * j + 1, :], pe_ps[32 * j:32 * j + 1, :])
            nc.sync.dma_start(out=outa[:, :], in_=res_a[:])
            nc.sync.dma_start(out=outb[:, :], in_=res_b[0:97:32, :])
    nc.finalize()
    return nc


def _shard_inputs(x1, x2, x3, share_feature, Wc1, Wc2, Wc4, Ws3, lin_b,
                  idx_h, idx_w, nkpe=NKPE, fd=FD):
    xc1 = _crop_gather(x1, idx_h[0], idx_w[0])            # [64, 1024, 49]
    xc2 = _crop_gather(x2, idx_h[1], idx_w[1])
    xc4 = _crop_gather(share_feature, idx_h[3], idx_w[3])
    x3f = x3.reshape(NB, 1280, 784)

    npad = fd - FCROP
    in_maps = []
    for m in range(NCORES):
        cs = slice(m * 128, (m + 1) * 128)
        cs3 = slice(m * 160, (m + 1) * 160)
        x3s = x3f[:, cs3].reshape(NB, 128, F3)            # [64, 128, 980]
        w3s = Ws3[cs3].reshape(128, F3)                   # [128, 980]

        xaf = np.concatenate([
            xc1[:, cs], xc2[:, cs], xc4[:, cs],
            np.zeros((NB, 128, npad), np.float32),
        ], axis=2)                                        # [64, 128, FD]
        # pack: batch b = NPACK*g+q, channel c = NPART*k+r ->
        # xa[NPART*q+r, g, k*FD+f] = xaf[NPACK*g+q, NPART*k+r, f]
        xa = xaf.reshape(NG, NPACK, NPACK, NPART, fd).transpose(1, 3, 0, 2, 4)
        xa = np.ascontiguousarray(
            xa.reshape(128, NG, NPACK * fd), dtype=np.float16)
        xb = np.ascontiguousarray(
            x3s[:, :, :nkpe].transpose(1, 2, 0), dtype=np.float16)

        waf = np.concatenate([
            Wc1[cs], Wc2[cs], Wc4[cs],
            np.zeros((128, npad), np.float32),
        ], axis=1) * W_SCALE                              # [128, FD]
        # wa[NPART*q+r, k*FD+f] = waf[NPART*k+r, f]  (independent of q)
        wav = np.tile(
            waf.reshape(NPACK, NPART, fd).transpose(1, 0, 2).reshape(NPART, -1),
            (NPACK, 1))
        wbv = w3s[:, :nkpe] * W_SCALE                     # [128, NKPE]

        selv = np.zeros((128, NPACK), np.float32)
        for q in range(NPACK):
            selv[NPART * q:NPART * (q + 1), q] = 1.0
        in_maps.append({
            'xa': xa,
            'xb': xb,
            'wa': np.ascontiguousarray(wav, dtype=np.float16),
            'wb': np.ascontiguousarray(wbv, dtype=np.float16),
            'seli': selv,
        })
    return in_maps


def _combine(results, lin_b):
    """Host-side finish: per-core partials -> [64, 1] fp32 output."""
    tot = np.zeros(NB, np.float64)
    for r in results:
        a = np.asarray(r['outa'], np.float64)             # [NPACK, NG] (q, g)
        b = np.asarray(r['outb'], np.float64).sum(axis=0)  # strips -> [64]
        tot += a.T.ravel() + b                            # b = NPACK*g+q order
    tot = tot / W_SCALE + float(lin_b[0])
    return tot.astype(np.float32).reshape(NB, 1)


def _ensure_ntff_hook():
    """Make `trace=True` (e.g. BASS_TRACE=1) work under axon even when the
    image's antenv package lacks axon_hooks: register an equivalent module
    backed by the ctypes NTFF hook from trn_agent_boot."""
    import sys
    import types
    try:
        import antenv.axon_hooks  # noqa: F401
        return
    except Exception:
        pass
    try:
        from trn_agent_boot import trn_boot
        hook = trn_boot._ntff_profile_via_ctypes('/opt/axon/libaxon_pjrt.so')
        mod = types.ModuleType('antenv.axon_hooks')
        mod.get_axon_ntff_profile_hook = lambda: hook
        mod.set_axon_ntff_profile_hook = lambda h: None
        sys.modules['antenv.axon_hooks'] = mod
    except Exception:
        pass


def kernel(x1, x2, x3, share_feature, c_w, conv3d_w, lin_w, lin_b,
           idx_h, idx_w):
    x1, x2, x3 = np.asarray(x1), np.asarray(x2), np.asarray(x3)
    share_feature = np.asarray(share_feature)
    c_w, conv3d_w = np.asarray(c_w), np.asarray(conv3d_w)
    lin_w, lin_b = np.asarray(lin_w), np.asarray(lin_b)
    idx_h, idx_w = np.asarray(idx_h), np.asarray(idx_w)
    _ensure_ntff_hook()
    Wc1, Wc2, Wc4, Ws3 = _build_fold(c_w, conv3d_w, lin_w, lin_b,
                                     idx_h, idx_w)
    in_maps = _shard_inputs(x1, x2, x3, share_feature,
                            Wc1, Wc2, Wc4, Ws3, lin_b, idx_h, idx_w)
    nc = _build_bass()
    res = run_bass_kernel_spmd(nc, in_maps, core_ids=list(range(NCORES)))
    return _combine(res.results, lin_b)
